# revision 3
# baseline (speedup 1.0000x reference)
"""ColBERT late-interaction scoring kernel for Trainium2 (Bass/Tile), v5.

score_b = sum_q max_k (2*Q@D^T - ||q||^2 - ||d||^2)
        = 2 * sum_q max_k (qd[q,k] - 0.5*dsq[k])  -  ||Q_b||_F^2

Sharding: data-parallel over batch B=128 across 8 NeuronCores (16 each).

v3 = v2 with the bias matmul FUSED into the main matmul via fp8 DoubleRow
(contraction = 2 x 128 planes):
  plane 0: lhsT = QT q-tile (fp8),  rhs = DT_b   (fp8)   -> qd[q,k]
  plane 1: lhsT = -0.5 const (fp8), rhs = SQ_b   (fp8)   -> -0.5*dsq[k]
One matmul per (batch, q-tile): 64 matmuls + 64 ldweights per rep instead of
v2's 128+128.  qsq stays bf16 (computed from the bf16 transpose).
"""

import numpy as np

B, LQ, LD, D = 128, 512, 512, 128
N_CORES = 8
BPC = B // N_CORES  # batches per core
NT = LQ // 128      # q tiles per batch

_compiled = {}


def _split_multi_waits(nc):
    """This container's walrus accepts only ONE sem-wait per instruction
    (setupSyncWait: 'Too many sync wait commands'). Tile's wait assignment
    emits multi-wait instructions, so split: every extra wait moves onto a
    dedicated NoOp inserted just before the instruction on the same engine.
    Engine program order makes this semantically identical."""
    import concourse.mybir as mybir

    for f in nc.m.functions:
        for blk in f.blocks:
            il = blk.instructions
            i = 0
            while i < len(il):
                inst = il[i]
                si = inst.sync_info
                waits = list(si.on_wait) if si and si.on_wait else []
                if len(waits) > 1:
                    for w in waits[:-1]:
                        nop = mybir.InstNoOp(
                            name=nc.get_next_instruction_name(), ins=[], outs=[]
                        )
                        nop.engine = inst.engine
                        nop.sync_info = mybir.SyncInfo(on_wait=[w], on_update=[])
                        il.insert(i, nop)
                        i += 1
                    inst.sync_info = mybir.SyncInfo(
                        on_wait=[waits[-1]], on_update=si.on_update
                    )
                i += 1


def _drop_ldweights(nc):
    """The tile legalizer splits every Matmult into Ldweights + Matmult.
    In this environment instruction dispatch is the dominant cost, so merge
    them back: delete each sync-free InstLdweights that immediately precedes
    its Matmult (the Matmult carries the weights AP in ins[1] already and
    walrus lowers a lone Matmult as self-loading)."""
    import concourse.mybir as mybir

    for f in nc.m.functions:
        for blk in f.blocks:
            il = blk.instructions
            keep = []
            for inst in il:
                if inst.opcode == "Ldweights":
                    si = inst.sync_info
                    if not si or (not si.on_wait and not si.on_update):
                        continue
                keep.append(inst)
            blk.instructions = keep


def _build(reps: int = 1):
    import concourse.bass as bass
    import concourse.mybir as mybir
    import concourse.tile as tile

    nc = bass.Bass()
    f32 = mybir.dt.float32
    bf16 = mybir.dt.bfloat16
    fp8 = mybir.dt.float8e4

    qe = nc.dram_tensor("qe", [BPC, LQ, D], f32, kind="ExternalInput")
    de = nc.dram_tensor("de", [BPC, LD, D], f32, kind="ExternalInput")
    out = nc.dram_tensor("out", [1, BPC], f32, kind="ExternalOutput")

    with tile.TileContext(nc) as tc:
        with (
            tc.tile_pool(name="consts", bufs=1) as cpool,
            tc.tile_pool(name="work", bufs=1) as wpool,
            tc.tile_pool(name="ps", bufs=2, space="PSUM") as pspool,
        ):
            ones_col = cpool.tile([128, 1], f32)
            nc.gpsimd.memset(ones_col, 1.0)

            # QTc8: plane 0 = QT_all (fp8, rewritten per rep); plane 1 = -0.5
            # constant (memset once, never touched again).
            QTc8 = cpool.tile([128, 2, BPC * LQ], fp8)
            nc.gpsimd.memset(QTc8[:, 1, :], -0.5)
            DSQc8 = cpool.tile([128, 2, BPC * LD], fp8)

            MX = cpool.tile([128, BPC * NT], f32)

            for rep in range(reps):
                # 1. loads (q = 4p + t layout)
                qf = wpool.tile([128, BPC, NT, 128], f32, tag="qf")
                df = wpool.tile([128, BPC, NT, 128], f32, tag="df")
                nc.sync.dma_start(qf, qe.rearrange("b (p t) d -> p b t d", t=NT))
                nc.sync.dma_start(df, de.rearrange("b (p t) d -> p b t d", t=NT))

                # 2. gpsimd cast -> bf16, natural layout [p, (b t d)]
                qb2 = wpool.tile([128, BPC, NT, 128], bf16, tag="qb2")
                db2 = wpool.tile([128, BPC, NT, 128], bf16, tag="db2")
                nc.gpsimd.tensor_copy(qb2, qf)
                nc.gpsimd.tensor_copy(db2, df)

                # 3. xbar transposes (2-byte dtype only). Semantics (verified in
                # CoreSim): out[f % 128, f // 128, p] = in[p, f].  With f ordered
                # (b, t, d): out[d, b*NT + t, p] -> QT_all[d, b*512 + t*128 + p].
                QTx = wpool.tile([128, BPC * NT, 128], bf16, tag="QTx")
                DTx = wpool.tile([128, BPC * NT, 128], bf16, tag="DTx")
                nc.sync.dma_start_transpose(
                    QTx, qb2.rearrange("p b t d -> p (b t d)")
                )
                nc.sync.dma_start_transpose(
                    DTx, db2.rearrange("p b t d -> p (b t d)")
                )
                QT_all = QTx.rearrange("d g p -> d (g p)")  # [128, 8192] bf16
                DT_all = DTx.rearrange("d g p -> d (g p)")

                # 4. fp8 packs + squares
                nc.gpsimd.tensor_copy(QTc8[:, 0, :], QT_all)
                nc.gpsimd.tensor_copy(DSQc8[:, 0, :], DT_all)
                # SQ plane: fp8(DT^2), from bf16 DT (scalar engine square)
                nc.scalar.activation(
                    DSQc8[:, 1, :], DT_all, mybir.ActivationFunctionType.Square
                )
                # qsq from bf16 (accuracy): QSQ = DT-accurate square of QT
                QSQ_all = wpool.tile([128, BPC * LQ], bf16, tag="QSQ")
                nc.scalar.activation(
                    QSQ_all, QT_all, mybir.ActivationFunctionType.Square
                )
                qsqd = cpool.tile([128, BPC], f32)
                nc.vector.reduce_sum(
                    qsqd,
                    QSQ_all.rearrange("d (b k) -> d b k", b=BPC),
                    axis=mybir.AxisListType.X,
                )

                # 5. main loop: one DoubleRow matmul per (b, t); 2 batches
                # per psum generation (all 8 banks), ONE reduce per 2 batches:
                # the PE<->DVE sync round cost dominates, so minimize rounds.
                for bb in range(BPC // 2):
                    pst = pspool.tile([128, 2, NT, LD], f32, tag="pst", bufs=1)
                    for i in range(2):
                        b = bb * 2 + i
                        rhs = DSQc8[:, :, b * LD : (b + 1) * LD]
                        for t in range(NT):
                            nc.tensor.matmul(
                                pst[:, i, t, :],
                                lhsT=QTc8[:, :, b * LQ + t * 128 : b * LQ + (t + 1) * 128],
                                rhs=rhs,
                                start=True, stop=True,
                                perf_mode=mybir.MatmulPerfMode.DoubleRow,
                            )
                    nc.vector.reduce_max(
                        MX[:, bb * 2 * NT : (bb + 1) * 2 * NT], pst,
                        axis=mybir.AxisListType.X,
                    )

                # 6. endgame
                msum = cpool.tile([128, BPC], f32)
                nc.vector.reduce_sum(
                    msum,
                    MX.rearrange("p (b t) -> p b t", t=NT),
                    axis=mybir.AxisListType.X,
                )
                sc = cpool.tile([128, BPC], f32)
                nc.vector.scalar_tensor_tensor(
                    sc, msum, 2.0, qsqd,
                    op0=mybir.AluOpType.mult, op1=mybir.AluOpType.subtract,
                )
                ps_s = pspool.tile([1, BPC], f32, tag="pst", bufs=1)
                nc.tensor.matmul(ps_s, lhsT=ones_col, rhs=sc, start=True, stop=True)
                score = cpool.tile([1, BPC], f32)
                nc.vector.tensor_copy(score, ps_s)
                nc.sync.dma_start(out[:, :], score)

    _split_multi_waits(nc)
    return nc


def kernel(query_embedding: np.ndarray, document_embedding: np.ndarray) -> np.ndarray:
    from concourse.bass_utils import run_bass_kernel_spmd

    if "nc" not in _compiled:
        _compiled["nc"] = _build()
    nc = _compiled["nc"]

    qe = np.ascontiguousarray(query_embedding, dtype=np.float32)
    de = np.ascontiguousarray(document_embedding, dtype=np.float32)
    in_maps = [
        {"qe": qe[c * BPC : (c + 1) * BPC], "de": de[c * BPC : (c + 1) * BPC]}
        for c in range(N_CORES)
    ]
    res = run_bass_kernel_spmd(nc, in_maps, core_ids=list(range(N_CORES)))
    return np.concatenate(
        [res.results[c]["out"].reshape(BPC) for c in range(N_CORES)]
    ).astype(np.float32)
